# revision 27
# baseline (speedup 1.0000x reference)
"""Trainium2 Bass kernel for nn_MultiHeadAttentionQuantum.

Math (verified vs reference):
  The per-token quantum feature map RX(x+theta) -> CNOT ring -> <Z_w>
  collapses to products of cosines. With u_w = cos(x_w + theta_w):
      q_0 = u1*u2*...*u7
      q_w = u0*u1*...*uw   (w = 1..7)
  Then per batch: attn = softmax(q @ q.T / sqrt(2)); out = attn @ q;
  out' = swapaxes(out,1,2).reshape(S,8); y = out' @ Wc.T + b.

Low-rank softmax (Nystrom): the Gram kernel K(a,b) = exp(a.b/sqrt2) on
the realized 8-dim q-manifold has fast eigendecay, so
    K ~= Phi W Phi^T,  Phi = exp(q @ Z^T / sqrt2),  W = (K_ZZ + eps I)^-1
with m=128 landmarks Z (k-means centers of the token q-cloud plus the
top-norm tokens, where exp is largest). Host picks Z/W from the inputs
(cheap numpy); the device computes
    num = Phi W (Phi^T q9)   (q9 = [q | 1] so col 8 is the softmax denom)
    out = num[:, :8] / num[:, 8:9]
This cuts exp count from S^2 to S*m per batch (ACT was the roofline)
and PE matmul work by a similar factor. Validated rel-err ~3.5e-3
(gate 2e-2) including fp16 staging; W is computed from the fp16-snapped
landmarks so device arithmetic is consistent with it.

Sharding: data-parallel over batch: 16 batches -> 8 cores x 2 batches.

Per-core per-batch device pipeline (P=128, T=32 chunks, m=128):
  phase Q: DMA x p-major (token s = 32p + t), theta+pi/2 broadcast add,
    range-reduce mod 2pi, u = ACT Sin, 13 strided DVE muls -> q fp32
    (pitch 9), fp16 copy -> q9h [128, T*128] (pitch 128, col 8 of each
    group = ones), XBAR DMA-transpose -> qTS [128, 32, 128]: feature w
    of token 32p+c sits at partition w, col-block c (PE operands only
    address base partition 0 this way).
  G': 32 matmuls Z8[8,128] x qTS-strip[8,128] -> PSUM [128,1024] tiles,
    ACT Exp -> PhiT [128 m, S] fp16 (landmark-major, col 128c+p =
    token 32p+c); XBAR -> Phi [128, 32, 128] token-major.
  A:  32 matmuls Phi-chunk[128tok,128m] x q9h-chunk[128,9] -> PSUM
    A [128 m, 9]; B = (W/4096) @ A (fp32 matmul) -> Bh fp16.
  numT: 32 matmuls PhiT-chunk[128,128] x Bh[128,9] -> PSUM [128, 32*9];
    col 8 of each 9-group = softmax denominator (scaled).
  normalize: DVE reciprocal + one broadcast multiply -> oY [128, 256]
    fp16 packed k-major (col 32k+c).
  combine: with the ABSOLUTE token order (token 128c+p on partition p,
    set up by the x-load rearrange), token%8 sits on partition%8, so
    the reference's swapaxes+reshape+combine collapses to ONE matmul
    against a constant block-diagonal W128 (16 copies of Wc^T):
      y[512k+16c+a, j] = sum_e Wc[j,e] * o(8*(16c+a)+e, k)
    = (W128^T @ oY)[(a,j), 32k+c].  Then bias via a per-partition
    tensor_scalar add, 2 PE transposes, and a fully CONTIGUOUS y store
    (y_flat[128n + r], n = col, r = (a,j)).  No DRAM scratch roundtrip,
    no gather DMAs.
  The two batches' stages are interleaved so PE never sits behind DMA,
  and ACT sees sin,sin,exp,...: 2 act-table loads total.
"""

import numpy as np

import concourse.bass as bass
import concourse.bacc as bacc
import concourse.tile as tile
from concourse.masks import make_identity
from concourse import mybir
from concourse.bass import broadcast_tensor_aps
from concourse._compat import with_exitstack

F32 = mybir.dt.float32
F16 = mybir.dt.float16
AF = mybir.ActivationFunctionType
P = 128
E = 8
E9 = 9
PITCH = 128

M = 128                       # Nystrom landmarks
NTOP = 64                     # landmarks taken from top-norm tokens
KM_ITERS = 12
EPS = 2e-3                    # ridge on K_ZZ
SQ = np.float64(1.0 / np.sqrt(2.0))


@with_exitstack
def _body(ctx, tc, x_in, thp, z8, wp, w128, bj, y, S, NB):
    nc = tc.nc
    T = S // P                 # token chunks per batch (32)

    const = ctx.enter_context(tc.tile_pool(name="const", bufs=1))
    qpool = ctx.enter_context(tc.tile_pool(name="qdata", bufs=1))
    work = ctx.enter_context(tc.tile_pool(name="work", bufs=2))
    g_ps = ctx.enter_context(tc.tile_pool(name="g_ps", bufs=2, space="PSUM"))
    u_ps = ctx.enter_context(tc.tile_pool(name="u_ps", bufs=3, space="PSUM"))

    thp_sb = const.tile([P, E], F32)
    z8_sb = const.tile([P, M], F16)
    w128_sb = const.tile([P, P], F16)
    bj_sb = const.tile([P, 1], F32)
    wp_sb = const.tile([P, M], F32)
    identh = const.tile([P, P], F16)
    make_identity(nc, identh[:])

    def load_consts():
        nc.sync.dma_start(thp_sb[:], thp[:])
        nc.sync.dma_start(z8_sb[:], z8[:])
        nc.sync.dma_start(w128_sb[:], w128[:])
        nc.sync.dma_start(bj_sb[:], bj[:])
        nc.sync.dma_start(wp_sb[:], wp[:])

    q9h = [qpool.tile([P, T * PITCH], F16, name=f"q9h{b}") for b in range(NB)]
    qTS = [qpool.tile([P, T * PITCH], F16, name=f"qTS{b}") for b in range(NB)]
    phiT = [qpool.tile([P, S], F16, name=f"phiT{b}") for b in range(NB)]
    phi = [qpool.tile([P, S], F16, name=f"phi{b}") for b in range(NB)]
    bsb = [qpool.tile([P, E9], F16, name=f"bsb{b}") for b in range(NB)]
    oY = [qpool.tile([P, T * E], F16, name=f"oY{b}") for b in range(NB)]
    ysb = [qpool.tile([P, T * E], F32, name=f"ysb{b}") for b in range(NB)]
    acc_sb = [qpool.tile([P, 4], F32, name=f"acc{b}") for b in range(NB)]

    # ------------- phase Q: quantum features (both batches fused) --------
    def load_x():
        # absolute token order: token 128t+p lands on partition p, col t
        xs = work.tile([P, NB * T * E], F32, tag="xs")
        xs3 = xs.rearrange("p (t w) -> p t w", w=E)
        tq = T // 4
        engs = [nc.sync, nc.scalar, nc.gpsimd, nc.scalar]
        for b in range(NB):
            xb = x_in[b].rearrange("(t p) w -> p t w", p=P)
            for s in range(4):
                engs[s].dma_start(
                    xs3[:, b * T + s * tq : b * T + (s + 1) * tq],
                    xb[:, s * tq : (s + 1) * tq],
                )
        return xs

    def phase_q(xs):
        T2 = NB * T
        x3 = xs.rearrange("p (t w) -> p t w", w=E)
        ph = work.tile([P, T2 * E], F32, tag="ph")
        p3 = ph.rearrange("p (t w) -> p t w", w=E)
        th3 = thp_sb.rearrange("p (o w) -> p o w", o=1)
        bx, bt = broadcast_tensor_aps(x3[:, :, :], th3[:, :, :])
        nc.vector.tensor_add(p3[:, :, :], bx, bt)
        # range-reduce ph mod 2*pi into [-pi, pi] (Sin spline domain)
        MAGIC = 12582912.0  # 1.5 * 2**23
        TWO_PI = 6.283185307179586
        rt = work.tile([P, T2 * E], F32, tag="rt")
        nc.vector.tensor_scalar(
            rt[:], ph[:], 1.0 / TWO_PI, MAGIC, mybir.AluOpType.mult, mybir.AluOpType.add
        )
        nc.vector.tensor_scalar(
            rt[:], rt[:], MAGIC, -TWO_PI, mybir.AluOpType.subtract, mybir.AluOpType.mult
        )
        nc.vector.tensor_add(ph[:], ph[:], rt[:])
        us = work.tile([P, T2 * E], F32, tag="us")
        nc.scalar.activation(us[:], ph[:], AF.Sin)
        u3 = us.rearrange("p (t w) -> p t w", w=E)
        q = work.tile([P, T2 * E9], F32, tag="q9f")
        q3 = q.rearrange("p (t e) -> p t e", e=E9)
        nc.vector.tensor_mul(q3[:, :, 1], u3[:, :, 0], u3[:, :, 1])
        for w in range(2, E):
            nc.vector.tensor_mul(q3[:, :, w], q3[:, :, w - 1], u3[:, :, w])
        nc.vector.tensor_mul(q3[:, :, 0], u3[:, :, 1], u3[:, :, 2])
        for w in range(3, E):
            nc.vector.tensor_mul(q3[:, :, 0], q3[:, :, 0], u3[:, :, w])
        return q3

    def phase_q2(b, q3):
        qh3 = q9h[b].rearrange("p (t e) -> p t e", e=PITCH)
        nc.vector.tensor_copy(qh3[:, :, 0:E], q3[:, b * T : (b + 1) * T, 0:E])
        # XBAR transpose: qTS[p, c, j] = q9h[j, 128*c + p]
        qt3 = qTS[b].rearrange("p (c j) -> p c j", j=P)
        half = T * PITCH // 2
        nc.sync.dma_start(qt3[:, 0 : T // 2], q9h[b][:, 0:half], transpose=True)
        nc.sync.dma_start(qt3[:, T // 2 :], q9h[b][:, half:], transpose=True)

    # ---------------- attention via Nystrom ------------------------------
    def gprime(b):
        qv = qTS[b].rearrange("p (c j) -> p c j", j=P)
        for g in range(4):
            gp = g_ps.tile([P, 2 * 512], F32, tag="gp")
            for cc in range(E):
                c = E * g + cc
                nc.tensor.matmul(
                    gp[:, cc * P : (cc + 1) * P],
                    z8_sb[0:E, :],
                    qv[0:E, c, :],
                    start=True,
                    stop=True,
                )
            nc.scalar.activation(
                phiT[b][:, g * 1024 : (g + 1) * 1024],
                gp[:],
                AF.Exp,
                accum_out=acc_sb[b][:, g : g + 1],
            )
        pv = phi[b].rearrange("p (c m) -> p c m", m=M)
        tq = T // 4
        for s in range(4):
            nc.sync.dma_start(
                pv[:, s * tq : (s + 1) * tq],
                phiT[b][:, s * tq * P : (s + 1) * tq * P],
                transpose=True,
            )

    def a_b_step(b):
        qh3 = q9h[b].rearrange("p (t e) -> p t e", e=PITCH)
        pv = phi[b].rearrange("p (c m) -> p c m", m=M)
        ap = u_ps.tile([P, 512], F32, tag="u")
        for c in range(T):
            nc.tensor.matmul(
                ap[:, 0:E],
                pv[:, c, :],
                qh3[:, c, 0:E],
                start=(c == 0),
                stop=(c == T - 1),
            )
        as_sb = work.tile([P, E9], F32, tag="as")
        nc.vector.tensor_copy(as_sb[:, 0:E], ap[:, 0:E])
        a01 = work.tile([P, 2], F32, tag="a01")
        nc.vector.tensor_add(a01[:, 0:1], acc_sb[b][:, 0:1], acc_sb[b][:, 1:2])
        nc.vector.tensor_add(a01[:, 1:2], acc_sb[b][:, 2:3], acc_sb[b][:, 3:4])
        nc.vector.tensor_add(as_sb[:, 8:9], a01[:, 0:1], a01[:, 1:2])
        bp = u_ps.tile([P, 512], F32, tag="u")
        nc.tensor.matmul(bp[:, 0:E9], wp_sb[:], as_sb[:], start=True, stop=True)
        nc.vector.tensor_copy(bsb[b][:], bp[:, 0:E9])

    def numt_norm(b):
        nt = u_ps.tile([P, 512], F32, tag="u")
        for c in range(T):
            nc.tensor.matmul(
                nt[:, c * E9 : (c + 1) * E9],
                phiT[b][:, c * P : (c + 1) * P],
                bsb[b][:],
                start=True,
                stop=True,
            )
        nt3 = nt[:, 0 : T * E9].rearrange("p (t e) -> p t e", e=E9)
        rec = work.tile([P, T], F32, tag="rec")
        nc.vector.reciprocal(rec[:], nt3[:, :, 8])
        rec3 = rec.rearrange("p (t o) -> p t o", o=1)
        oy3 = oY[b].rearrange("p (k c) -> p c k", k=E)
        bn, br = broadcast_tensor_aps(nt3[:, :, 0:E], rec3[:, :, :])
        nc.vector.tensor_mul(oy3[:, :, :], bn, br)

    def combine(b):
        cw = u_ps.tile([P, 512], F32, tag="u")
        nc.tensor.matmul(cw[:, 0 : T * E], w128_sb[:], oY[b][:], start=True, stop=True)
        oc = work.tile([P, T * E], F16, tag="oc")
        nc.vector.tensor_scalar_add(oc[:], cw[:, 0 : T * E], bj_sb[:, 0:1])
        tp = u_ps.tile([P, 512], F16, tag="u")
        for t in range(2):
            nc.tensor.transpose(
                tp[:, t * P : (t + 1) * P], oc[:, t * P : (t + 1) * P], identh[:]
            )
        nc.vector.tensor_copy(ysb[b][:], tp[:, 0 : T * E])
        yv2 = y[b].rearrange("(n q) j -> n (q j)", q=P // E)
        yo = ysb[b]
        nc.scalar.dma_start(yv2[0:P, :], yo[:, 0:P])
        nc.scalar.dma_start(yv2[P : 2 * P, :], yo[:, P : 2 * P])

    xs = load_x()
    load_consts()
    q3 = phase_q(xs)
    phase_q2(0, q3)
    phase_q2(1, q3)
    gprime(0)
    gprime(1)
    a_b_step(0)
    numt_norm(0)
    a_b_step(1)
    combine(0)
    numt_norm(1)
    combine(1)


def build_nc(S=4096, NB=2):
    nc = bacc.Bacc(None, target_bir_lowering=False)
    x_in = nc.dram_tensor("x", (NB, S, E), F32, kind="ExternalInput")
    thp = nc.dram_tensor("thp", (P, E), F32, kind="ExternalInput")
    z8 = nc.dram_tensor("z8", (P, M), F16, kind="ExternalInput")
    wp = nc.dram_tensor("wp", (P, M), F32, kind="ExternalInput")
    w128 = nc.dram_tensor("w128", (P, P), F16, kind="ExternalInput")
    bj = nc.dram_tensor("bj", (P, 1), F32, kind="ExternalInput")
    y = nc.dram_tensor("y", (NB, S, E), F32, kind="ExternalOutput")
    with tile.TileContext(nc) as tc:
        _body(tc, x_in[:], thp[:], z8[:], wp[:], w128[:], bj[:], y[:], S, NB)
    nc.compile()
    return nc


def _qfeat(x, theta):
    u = np.cos(np.asarray(x, np.float32) + np.asarray(theta, np.float32))
    q = np.empty_like(u)
    q[..., 0] = np.prod(u[..., 1:], axis=-1)
    c = u[..., 0].copy()
    for w in range(1, E):
        c = c * u[..., w]
        q[..., w] = c
    return q


def _landmarks(x, theta):
    qa = _qfeat(x, theta).reshape(-1, E).astype(np.float32)
    r = np.random.default_rng(20260809)
    pool = qa[r.choice(len(qa), min(16384, len(qa)), replace=False)]
    mk = M - NTOP
    C = pool[r.choice(len(pool), mk, replace=False)].copy()
    for _ in range(KM_ITERS):
        lab = np.empty(len(pool), np.int64)
        for i in range(0, len(pool), 8192):
            dd = ((pool[i : i + 8192, None, :] - C[None, :, :]) ** 2).sum(-1)
            lab[i : i + 8192] = dd.argmin(1)
        for k in range(mk):
            s = lab == k
            if s.any():
                C[k] = pool[s].mean(0)
    nrm = (qa ** 2).sum(1)
    top = qa[np.argpartition(nrm, -NTOP)[-NTOP:]]
    Z = np.concatenate([C, top], 0).astype(np.float32)
    # snap to the fp16 values the device will use, derive W consistently
    zs16 = (Z * np.float32(SQ)).astype(np.float16)
    zeff = (zs16.astype(np.float64)) / SQ
    kzz = np.exp((zeff @ zeff.T) * SQ)
    W = np.linalg.inv(kzz + EPS * np.eye(M))
    W = (W + W.T) * 0.5
    return zs16, (W / 4096.0).astype(np.float32)


def host_inputs(x, theta, w_combine, b_combine):
    zs16, wp = _landmarks(x, theta)
    thp = np.tile(
        (np.asarray(theta, np.float32) + np.float32(np.pi / 2))[None, :], (P, 1)
    ).astype(np.float32)
    z8 = np.zeros((P, M), np.float16)
    for s in range(4):
        z8[32 * s : 32 * s + E, :] = zs16.T
    w128 = np.zeros((P, P), np.float16)
    wct = np.asarray(w_combine, np.float32).T.astype(np.float16)
    for a in range(P // E):
        w128[E * a : E * a + E, E * a : E * a + E] = wct
    bj = np.tile(np.asarray(b_combine, np.float32), P // E)[:, None].astype(np.float32)
    return thp, z8, wp, w128, bj


_NC_CACHE = {}


def _prepare(x, theta, w_combine, b_combine):
    x = np.asarray(x, np.float32)
    B, S, _ = x.shape
    NCORES = 8
    NB = B // NCORES
    key = (S, NB)
    if key not in _NC_CACHE:
        _NC_CACHE[key] = build_nc(S=S, NB=NB)
    nc = _NC_CACHE[key]
    thp, z8, wp, w128, bj = host_inputs(x, theta, w_combine, b_combine)
    in_maps = [
        {
            "x": x[c * NB : (c + 1) * NB],
            "thp": thp,
            "z8": z8,
            "wp": wp,
            "w128": w128,
            "bj": bj,
        }
        for c in range(NCORES)
    ]
    return nc, in_maps


def kernel(x, theta, w_combine, b_combine):
    from concourse.bass_utils import run_bass_kernel_spmd

    nc, in_maps = _prepare(x, theta, w_combine, b_combine)
    res = run_bass_kernel_spmd(nc, in_maps, list(range(8))).results
    return np.concatenate([res[c]["y"] for c in range(8)], axis=0)


# revision 28
# speedup vs baseline: 1.0043x; 1.0043x over previous
"""Trainium2 Bass kernel for nn_MultiHeadAttentionQuantum.

Math (verified vs reference):
  The per-token quantum feature map RX(x+theta) -> CNOT ring -> <Z_w>
  collapses to products of cosines. With u_w = cos(x_w + theta_w):
      q_0 = u1*u2*...*u7
      q_w = u0*u1*...*uw   (w = 1..7)
  Then per batch: attn = softmax(q @ q.T / sqrt(2)); out = attn @ q;
  out' = swapaxes(out,1,2).reshape(S,8); y = out' @ Wc.T + b.

Low-rank softmax (Nystrom): the Gram kernel K(a,b) = exp(a.b/sqrt2) on
the realized 8-dim q-manifold has fast eigendecay, so
    K ~= Phi W Phi^T,  Phi = exp(q @ Z^T / sqrt2),  W = (K_ZZ + eps I)^-1
with m=128 landmarks Z (k-means centers of the token q-cloud plus the
top-norm tokens, where exp is largest). Host picks Z/W from the inputs
(cheap numpy); the device computes
    num = Phi W (Phi^T q9)   (q9 = [q | 1] so col 8 is the softmax denom)
    out = num[:, :8] / num[:, 8:9]
This cuts exp count from S^2 to S*m per batch (ACT was the roofline)
and PE matmul work by a similar factor. Validated rel-err ~3.5e-3
(gate 2e-2) including fp16 staging; W is computed from the fp16-snapped
landmarks so device arithmetic is consistent with it.

Sharding: data-parallel over batch: 16 batches -> 8 cores x 2 batches.

Per-core per-batch device pipeline (P=128, T=32 chunks, m=128):
  phase Q: DMA x p-major (token s = 32p + t), theta+pi/2 broadcast add,
    range-reduce mod 2pi, u = ACT Sin, 13 strided DVE muls -> q fp32
    (pitch 9), fp16 copy -> q9h [128, T*128] (pitch 128, col 8 of each
    group = ones), XBAR DMA-transpose -> qTS [128, 32, 128]: feature w
    of token 32p+c sits at partition w, col-block c (PE operands only
    address base partition 0 this way).
  G': 32 matmuls Z8[8,128] x qTS-strip[8,128] -> PSUM [128,1024] tiles,
    ACT Exp -> PhiT [128 m, S] fp16 (landmark-major, col 128c+p =
    token 32p+c); XBAR -> Phi [128, 32, 128] token-major.
  A:  32 matmuls Phi-chunk[128tok,128m] x q9h-chunk[128,9] -> PSUM
    A [128 m, 9]; B = (W/4096) @ A (fp32 matmul) -> Bh fp16.
  numT: 32 matmuls PhiT-chunk[128,128] x Bh[128,9] -> PSUM [128, 32*9];
    col 8 of each 9-group = softmax denominator (scaled).
  normalize: DVE reciprocal + one broadcast multiply -> oY [128, 256]
    fp16 packed k-major (col 32k+c).
  combine: with the ABSOLUTE token order (token 128c+p on partition p,
    set up by the x-load rearrange), token%8 sits on partition%8, so
    the reference's swapaxes+reshape+combine collapses to ONE matmul
    against a constant block-diagonal W128 (16 copies of Wc^T):
      y[512k+16c+a, j] = sum_e Wc[j,e] * o(8*(16c+a)+e, k)
    = (W128^T @ oY)[(a,j), 32k+c].  Then bias via a per-partition
    tensor_scalar add, 2 PE transposes, and a fully CONTIGUOUS y store
    (y_flat[128n + r], n = col, r = (a,j)).  No DRAM scratch roundtrip,
    no gather DMAs.
  The two batches' stages are interleaved so PE never sits behind DMA,
  and ACT sees sin,sin,exp,...: 2 act-table loads total.
"""

import numpy as np

import concourse.bass as bass
import concourse.bacc as bacc
import concourse.tile as tile
from concourse.masks import make_identity
from concourse import mybir
from concourse.bass import broadcast_tensor_aps
from concourse._compat import with_exitstack

F32 = mybir.dt.float32
F16 = mybir.dt.float16
AF = mybir.ActivationFunctionType
P = 128
E = 8
E9 = 9
PITCH = 128

M = 128                       # Nystrom landmarks
NTOP = 64                     # landmarks taken from top-norm tokens
KM_ITERS = 12
EPS = 2e-3                    # ridge on K_ZZ
SQ = np.float64(1.0 / np.sqrt(2.0))


@with_exitstack
def _body(ctx, tc, x_in, thp, z8, wp, w128, bj, y, S, NB):
    nc = tc.nc
    T = S // P                 # token chunks per batch (32)

    const = ctx.enter_context(tc.tile_pool(name="const", bufs=1))
    qpool = ctx.enter_context(tc.tile_pool(name="qdata", bufs=1))
    work = ctx.enter_context(tc.tile_pool(name="work", bufs=2))
    g_ps = ctx.enter_context(tc.tile_pool(name="g_ps", bufs=2, space="PSUM"))
    u_ps = ctx.enter_context(tc.tile_pool(name="u_ps", bufs=3, space="PSUM"))

    thp_sb = const.tile([P, E], F32)
    z8_sb = const.tile([P, M], F16)
    w128_sb = const.tile([P, P], F16)
    bj_sb = const.tile([P, 1], F32)
    wp_sb = const.tile([P, M], F32)
    identh = const.tile([P, P], F16)
    make_identity(nc, identh[:])

    def load_consts():
        nc.sync.dma_start(thp_sb[:], thp[:])
        nc.sync.dma_start(z8_sb[:], z8[:])
        nc.sync.dma_start(w128_sb[:], w128[:])
        nc.sync.dma_start(bj_sb[:], bj[:])
        nc.sync.dma_start(wp_sb[:], wp[:])

    q9h = [qpool.tile([P, T * PITCH], F16, name=f"q9h{b}") for b in range(NB)]
    qTS = [qpool.tile([P, T * PITCH], F16, name=f"qTS{b}") for b in range(NB)]
    phiT = [qpool.tile([P, S], F16, name=f"phiT{b}") for b in range(NB)]
    phi = [qpool.tile([P, S], F16, name=f"phi{b}") for b in range(NB)]
    bsb = [qpool.tile([P, E9], F16, name=f"bsb{b}") for b in range(NB)]
    oY = [qpool.tile([P, T * E], F16, name=f"oY{b}") for b in range(NB)]
    ysb = [qpool.tile([P, T * E], F32, name=f"ysb{b}") for b in range(NB)]
    acc_sb = [qpool.tile([P, 4], F32, name=f"acc{b}") for b in range(NB)]

    # ------------- phase Q: quantum features (both batches fused) --------
    def load_x():
        # absolute token order: token 128t+p lands on partition p, col t
        xs = work.tile([P, NB * T * E], F32, tag="xs")
        xs3 = xs.rearrange("p (t w) -> p t w", w=E)
        tq = T // 4
        engs = [nc.sync, nc.scalar, nc.gpsimd, nc.scalar]
        for b in range(NB):
            xb = x_in[b].rearrange("(t p) w -> p t w", p=P)
            for s in range(4):
                engs[s].dma_start(
                    xs3[:, b * T + s * tq : b * T + (s + 1) * tq],
                    xb[:, s * tq : (s + 1) * tq],
                )
        return xs

    def phase_q(xs):
        T2 = NB * T
        x3 = xs.rearrange("p (t w) -> p t w", w=E)
        ph = work.tile([P, T2 * E], F32, tag="ph")
        p3 = ph.rearrange("p (t w) -> p t w", w=E)
        th3 = thp_sb.rearrange("p (o w) -> p o w", o=1)
        bx, bt = broadcast_tensor_aps(x3[:, :, :], th3[:, :, :])
        nc.vector.tensor_add(p3[:, :, :], bx, bt)
        # range-reduce ph mod 2*pi into [-pi, pi] (Sin spline domain)
        MAGIC = 12582912.0  # 1.5 * 2**23
        TWO_PI = 6.283185307179586
        rt = work.tile([P, T2 * E], F32, tag="rt")
        nc.vector.tensor_scalar(
            rt[:], ph[:], 1.0 / TWO_PI, MAGIC, mybir.AluOpType.mult, mybir.AluOpType.add
        )
        nc.vector.tensor_scalar(
            rt[:], rt[:], MAGIC, -TWO_PI, mybir.AluOpType.subtract, mybir.AluOpType.mult
        )
        nc.vector.tensor_add(ph[:], ph[:], rt[:])
        us = work.tile([P, T2 * E], F32, tag="us")
        nc.scalar.activation(us[:], ph[:], AF.Sin)
        u3 = us.rearrange("p (t w) -> p t w", w=E)
        q = work.tile([P, T2 * E9], F32, tag="q9f")
        q3 = q.rearrange("p (t e) -> p t e", e=E9)
        nc.vector.tensor_mul(q3[:, :, 1], u3[:, :, 0], u3[:, :, 1])
        for w in range(2, E):
            nc.vector.tensor_mul(q3[:, :, w], q3[:, :, w - 1], u3[:, :, w])
        nc.vector.tensor_mul(q3[:, :, 0], u3[:, :, 1], u3[:, :, 2])
        for w in range(3, E):
            nc.vector.tensor_mul(q3[:, :, 0], q3[:, :, 0], u3[:, :, w])
        return q3

    def phase_q2(b, q3):
        qh3 = q9h[b].rearrange("p (t e) -> p t e", e=PITCH)
        nc.vector.tensor_copy(qh3[:, :, 0:E], q3[:, b * T : (b + 1) * T, 0:E])
        # XBAR transpose: qTS[p, c, j] = q9h[j, 128*c + p]
        qt3 = qTS[b].rearrange("p (c j) -> p c j", j=P)
        half = T * PITCH // 2
        nc.sync.dma_start(qt3[:, 0 : T // 2], q9h[b][:, 0:half], transpose=True)
        nc.sync.dma_start(qt3[:, T // 2 :], q9h[b][:, half:], transpose=True)

    # ---------------- attention via Nystrom ------------------------------
    def gprime(b):
        qv = qTS[b].rearrange("p (c j) -> p c j", j=P)
        for g in range(4):
            gp = g_ps.tile([P, 2 * 512], F32, tag="gp")
            for cc in range(E):
                c = E * g + cc
                nc.tensor.matmul(
                    gp[:, cc * P : (cc + 1) * P],
                    z8_sb[0:E, :],
                    qv[0:E, c, :],
                    start=True,
                    stop=True,
                )
            nc.scalar.activation(
                phiT[b][:, g * 1024 : (g + 1) * 1024],
                gp[:],
                AF.Exp,
                accum_out=acc_sb[b][:, g : g + 1],
            )
        pv = phi[b].rearrange("p (c m) -> p c m", m=M)
        nc.sync.dma_start(pv[:, 0 : T // 2], phiT[b][:, 0 : S // 2], transpose=True)
        nc.sync.dma_start(pv[:, T // 2 :], phiT[b][:, S // 2 :], transpose=True)

    def a_b_step(b):
        qh3 = q9h[b].rearrange("p (t e) -> p t e", e=PITCH)
        pv = phi[b].rearrange("p (c m) -> p c m", m=M)
        ap = u_ps.tile([P, 512], F32, tag="u")
        for c in range(T):
            nc.tensor.matmul(
                ap[:, 0:E],
                pv[:, c, :],
                qh3[:, c, 0:E],
                start=(c == 0),
                stop=(c == T - 1),
            )
        as_sb = work.tile([P, E9], F32, tag="as")
        nc.vector.tensor_copy(as_sb[:, 0:E], ap[:, 0:E])
        a01 = work.tile([P, 2], F32, tag="a01")
        nc.vector.tensor_add(a01[:, 0:1], acc_sb[b][:, 0:1], acc_sb[b][:, 1:2])
        nc.vector.tensor_add(a01[:, 1:2], acc_sb[b][:, 2:3], acc_sb[b][:, 3:4])
        nc.vector.tensor_add(as_sb[:, 8:9], a01[:, 0:1], a01[:, 1:2])
        bp = u_ps.tile([P, 512], F32, tag="u")
        nc.tensor.matmul(bp[:, 0:E9], wp_sb[:], as_sb[:], start=True, stop=True)
        nc.vector.tensor_copy(bsb[b][:], bp[:, 0:E9])

    def numt_norm(b):
        nt = u_ps.tile([P, 512], F32, tag="u")
        for c in range(T):
            nc.tensor.matmul(
                nt[:, c * E9 : (c + 1) * E9],
                phiT[b][:, c * P : (c + 1) * P],
                bsb[b][:],
                start=True,
                stop=True,
            )
        nt3 = nt[:, 0 : T * E9].rearrange("p (t e) -> p t e", e=E9)
        rec = work.tile([P, T], F32, tag="rec")
        nc.vector.reciprocal(rec[:], nt3[:, :, 8])
        rec3 = rec.rearrange("p (t o) -> p t o", o=1)
        oy3 = oY[b].rearrange("p (k c) -> p c k", k=E)
        bn, br = broadcast_tensor_aps(nt3[:, :, 0:E], rec3[:, :, :])
        nc.vector.tensor_mul(oy3[:, :, :], bn, br)

    def combine(b):
        cw = u_ps.tile([P, 512], F32, tag="u")
        nc.tensor.matmul(cw[:, 0 : T * E], w128_sb[:], oY[b][:], start=True, stop=True)
        oc = work.tile([P, T * E], F16, tag="oc")
        nc.vector.tensor_scalar_add(oc[:], cw[:, 0 : T * E], bj_sb[:, 0:1])
        tp = u_ps.tile([P, 512], F16, tag="u")
        for t in range(2):
            nc.tensor.transpose(
                tp[:, t * P : (t + 1) * P], oc[:, t * P : (t + 1) * P], identh[:]
            )
        nc.vector.tensor_copy(ysb[b][:], tp[:, 0 : T * E])
        yv2 = y[b].rearrange("(n q) j -> n (q j)", q=P // E)
        yo = ysb[b]
        nc.scalar.dma_start(yv2[0:P, :], yo[:, 0:P])
        nc.scalar.dma_start(yv2[P : 2 * P, :], yo[:, P : 2 * P])

    xs = load_x()
    load_consts()
    q3 = phase_q(xs)
    phase_q2(0, q3)
    phase_q2(1, q3)
    gprime(0)
    gprime(1)
    a_b_step(0)
    numt_norm(0)
    a_b_step(1)
    combine(0)
    numt_norm(1)
    combine(1)


def build_nc(S=4096, NB=2):
    nc = bacc.Bacc(None, target_bir_lowering=False)
    x_in = nc.dram_tensor("x", (NB, S, E), F32, kind="ExternalInput")
    thp = nc.dram_tensor("thp", (P, E), F32, kind="ExternalInput")
    z8 = nc.dram_tensor("z8", (P, M), F16, kind="ExternalInput")
    wp = nc.dram_tensor("wp", (P, M), F32, kind="ExternalInput")
    w128 = nc.dram_tensor("w128", (P, P), F16, kind="ExternalInput")
    bj = nc.dram_tensor("bj", (P, 1), F32, kind="ExternalInput")
    y = nc.dram_tensor("y", (NB, S, E), F32, kind="ExternalOutput")
    with tile.TileContext(nc) as tc:
        _body(tc, x_in[:], thp[:], z8[:], wp[:], w128[:], bj[:], y[:], S, NB)
    nc.compile()
    return nc


def _qfeat(x, theta):
    u = np.cos(np.asarray(x, np.float32) + np.asarray(theta, np.float32))
    q = np.empty_like(u)
    q[..., 0] = np.prod(u[..., 1:], axis=-1)
    c = u[..., 0].copy()
    for w in range(1, E):
        c = c * u[..., w]
        q[..., w] = c
    return q


def _landmarks(x, theta):
    qa = _qfeat(x, theta).reshape(-1, E).astype(np.float32)
    r = np.random.default_rng(20260809)
    pool = qa[r.choice(len(qa), min(16384, len(qa)), replace=False)]
    mk = M - NTOP
    C = pool[r.choice(len(pool), mk, replace=False)].copy()
    for _ in range(KM_ITERS):
        lab = np.empty(len(pool), np.int64)
        for i in range(0, len(pool), 8192):
            dd = ((pool[i : i + 8192, None, :] - C[None, :, :]) ** 2).sum(-1)
            lab[i : i + 8192] = dd.argmin(1)
        for k in range(mk):
            s = lab == k
            if s.any():
                C[k] = pool[s].mean(0)
    nrm = (qa ** 2).sum(1)
    top = qa[np.argpartition(nrm, -NTOP)[-NTOP:]]
    Z = np.concatenate([C, top], 0).astype(np.float32)
    # snap to the fp16 values the device will use, derive W consistently
    zs16 = (Z * np.float32(SQ)).astype(np.float16)
    zeff = (zs16.astype(np.float64)) / SQ
    kzz = np.exp((zeff @ zeff.T) * SQ)
    W = np.linalg.inv(kzz + EPS * np.eye(M))
    W = (W + W.T) * 0.5
    return zs16, (W / 4096.0).astype(np.float32)


def host_inputs(x, theta, w_combine, b_combine):
    zs16, wp = _landmarks(x, theta)
    thp = np.tile(
        (np.asarray(theta, np.float32) + np.float32(np.pi / 2))[None, :], (P, 1)
    ).astype(np.float32)
    z8 = np.zeros((P, M), np.float16)
    for s in range(4):
        z8[32 * s : 32 * s + E, :] = zs16.T
    w128 = np.zeros((P, P), np.float16)
    wct = np.asarray(w_combine, np.float32).T.astype(np.float16)
    for a in range(P // E):
        w128[E * a : E * a + E, E * a : E * a + E] = wct
    bj = np.tile(np.asarray(b_combine, np.float32), P // E)[:, None].astype(np.float32)
    return thp, z8, wp, w128, bj


_NC_CACHE = {}


def _prepare(x, theta, w_combine, b_combine):
    x = np.asarray(x, np.float32)
    B, S, _ = x.shape
    NCORES = 8
    NB = B // NCORES
    key = (S, NB)
    if key not in _NC_CACHE:
        _NC_CACHE[key] = build_nc(S=S, NB=NB)
    nc = _NC_CACHE[key]
    thp, z8, wp, w128, bj = host_inputs(x, theta, w_combine, b_combine)
    in_maps = [
        {
            "x": x[c * NB : (c + 1) * NB],
            "thp": thp,
            "z8": z8,
            "wp": wp,
            "w128": w128,
            "bj": bj,
        }
        for c in range(NCORES)
    ]
    return nc, in_maps


def kernel(x, theta, w_combine, b_combine):
    from concourse.bass_utils import run_bass_kernel_spmd

    nc, in_maps = _prepare(x, theta, w_combine, b_combine)
    res = run_bass_kernel_spmd(nc, in_maps, list(range(8))).results
    return np.concatenate([res[c]["y"] for c in range(8)], axis=0)
